# revision 30
# baseline (speedup 1.0000x reference)
"""Causal self-attention (B=2, T=2048, D=1024, H=16) on 8 Trainium2 NeuronCores.

Sharding: data-parallel on batch (2-way) x tensor-parallel on heads (4-way):
each core owns one batch's activations and 4 heads (256 channels) of the
QKV / output-projection weights.  Host pre-transposes x and packs all weight
shards into one [128, 8320] tensor laid out exactly as SBUF wants them, so
the whole input loads in ~3 DMAs with 32KB/16KB-per-partition descriptors:
  qT/kT = W[heads] @ x.T        (channels on partitions, T on free axis)
  v     = x @ Wv[heads].T       (T on partitions) + ones column (softmax sum)
  expST[j,t] = exp(0.125 * k_h q_h^T)   (kv-position on partitions)
  yT_aug = v_aug.T @ expST      (row 64 = softmax denominator)
  yT     = yT_aug[:64] * recip(denom) broadcast across partitions
  partial out = yT.T @ WpT[heads]  -> [T, D] partial per core, summed on host.
Causality: fully-masked 128-col j-blocks are skipped, partially-masked columns
sliced away, and one 128x128 triangular mask multiplies the diagonal block.
Softmax skips max-subtraction (scores are O(1) by construction).
The two heads of a pair sit at partition bases 0/64 so their K=64 score
matmuls run concurrently in separate PE row-groups.

Pipeline discipline (the point of this version): the PE must stream
back-to-back or the HAM clock-gate drops it to half clock.  So
  - attention starts as early as the first q/k/v tiles exist (~10us, not
    ~40us): the prelude computes only (q,k) for head-pair 0 of t-block 0
    plus v for the first kv block; the rest of QKV is dripped as filler,
  - att@V consumption LAGS the exp stream by 2 chunks so the PE never
    blocks on the Scalar engine; filler groups are dripped BEFORE each
    dependent att@V,
  - softmax normalization uses reciprocal_approx_fast straight off the
    PSUM denominator row + a gpsimd partition-broadcast (no DMA
    round-trips on the critical path),
  - the output projection is deferred into the exp-heavy late t-blocks and
    the final head-pair chases quarter-blocks so the tail stays dense.
"""
import sys, types

for _p in ("/opt/trn_rl_repo",):
    if _p not in sys.path:
        sys.path.append(_p)


def _install_ntff_hook():
    """Register the axon NTFF profile hook that container boot skips when
    antenv.axon_hooks is absent (needed only for profiled runs)."""
    if "antenv.axon_hooks" in sys.modules:
        return
    mod = types.ModuleType("antenv.axon_hooks")
    _h = [None]
    mod.set_axon_ntff_profile_hook = lambda h: _h.__setitem__(0, h)
    mod.get_axon_ntff_profile_hook = lambda: _h[0]
    sys.modules["antenv.axon_hooks"] = mod
    try:
        import antenv
        antenv.axon_hooks = mod
    except Exception:
        pass
    try:
        from trn_agent_boot.trn_boot import _ntff_profile_via_ctypes
        mod.set_axon_ntff_profile_hook(
            _ntff_profile_via_ctypes("/opt/axon/libaxon_pjrt.so"))
    except Exception:
        pass


_install_ntff_hook()

import numpy as np
import ml_dtypes

import concourse.tile as tile
from concourse import bacc, mybir, bass_utils

B, T, D, H = 2, 2048, 1024, 16
HD = 64
NHL = 4            # heads per core
C = NHL * HD       # 256 channels per core
DP = 1024          # contraction dim (biases added separately)
KC = DP // 128     # 8
P = 128
BF = mybir.dt.bfloat16
F32 = mybir.dt.float32
bf16 = ml_dtypes.bfloat16

N_CORES = 8
WARM = 22          # HAM pre-warm matmuls covering the input-DMA window
LAG = 2            # chunks the att@V stream trails the exp stream by
DEBUG = False      # add qt/kt/v/yt DRAM taps for numeric debugging

# packed weight layout offsets (bf16 columns per partition), ordered so the
# first DMA chunk carries exactly what attention chunk 0 needs
WQ0_OFF, WK0_OFF, TM_OFF = 0, 1024, 2048
WV_OFF, WQ1_OFF, WK1_OFF, WP_OFF = 2176, 4224, 5248, 6272
WPK_COLS = 8320


def build_graph():
    nc = bacc.Bacc("TRN2", target_bir_lowering=False, debug=False,
                   num_devices=N_CORES)
    xt_d = nc.dram_tensor("xt", [P, KC * T], BF, kind="ExternalInput").ap()
    wpk_d = nc.dram_tensor("wpk", [P, WPK_COLS], BF, kind="ExternalInput").ap()
    bqk_d = nc.dram_tensor("bqk", [P, 4], F32, kind="ExternalInput").ap()
    bv_d = nc.dram_tensor("bv", [1, C], F32, kind="ExternalInput").ap()
    out_d = nc.dram_tensor("out", [T, D], BF, kind="ExternalOutput").ap()
    if DEBUG:
        qt_o = nc.dram_tensor("qt_o", [P, 2 * T], BF, kind="ExternalOutput").ap()
        kt_o = nc.dram_tensor("kt_o", [P, 2 * T], BF, kind="ExternalOutput").ap()
        v_o = nc.dram_tensor("v_o", [P, 16 * NHL * (HD + 1)], BF,
                             kind="ExternalOutput").ap()
        yt_o = nc.dram_tensor("yt_o", [P, 2 * T], BF, kind="ExternalOutput").ap()
        rec_o = nc.dram_tensor("rec_o", [16, 512], F32, kind="ExternalOutput").ap()
        den_o = nc.dram_tensor("den_o", [16, 512], F32, kind="ExternalOutput").ap()
        yu_o = nc.dram_tensor("yu_o", [HD, 512], F32, kind="ExternalOutput").ap()

    Exp = mybir.ActivationFunctionType.Exp
    Ident = mybir.ActivationFunctionType.Identity

    with tile.TileContext(nc) as tc:
        with tc.tile_pool(name="sing", bufs=1) as sing, \
             tc.tile_pool(name="fill", bufs=1, space="PSUM") as fillps, \
             tc.tile_pool(name="stps", bufs=2, space="PSUM") as stps, \
             tc.tile_pool(name="ytps", bufs=2, space="PSUM") as ytps, \
             tc.tile_pool(name="esb", bufs=4) as esb, \
             tc.tile_pool(name="nrm", bufs=6) as nrm, \
             tc.tile_pool(name="osb", bufs=4) as osb:
            xt_sb = sing.tile([P, KC, T], BF)
            wpk_sb = sing.tile([P, WPK_COLS], BF)
            qt_sb = sing.tile([P, 2, T], BF)
            kt_sb = sing.tile([P, 2, T], BF)
            v_sb = sing.tile([P, 16, NHL, HD + 1], BF)
            yt_sb = sing.tile([P, 2, T], BF)
            bias_sb = sing.tile([P, 2, 2], F32)
            bv_row = sing.tile([1, C], F32)
            bvb_sb = sing.tile([P, C], F32)
            warm_sb = sing.tile([P, 512], BF)
            warm_out = sing.tile([1, 8], BF)

            wq_cc = [wpk_sb[:, WQ0_OFF:WQ0_OFF + 1024].rearrange(
                         "p (kc c) -> p kc c", kc=KC),
                     wpk_sb[:, WQ1_OFF:WQ1_OFF + 1024].rearrange(
                         "p (kc c) -> p kc c", kc=KC)]
            wk_cc = [wpk_sb[:, WK0_OFF:WK0_OFF + 1024].rearrange(
                         "p (kc c) -> p kc c", kc=KC),
                     wpk_sb[:, WK1_OFF:WK1_OFF + 1024].rearrange(
                         "p (kc c) -> p kc c", kc=KC)]
            wv_sb = wpk_sb[:, WV_OFF:WV_OFF + 2048].rearrange(
                "p (kc c) -> p kc c", kc=KC)
            wp_sb = wpk_sb[:, WP_OFF:WP_OFF + 2048].rearrange(
                "p (cc o) -> p cc o", cc=2)
            tm_sb = wpk_sb[:, TM_OFF:TM_OFF + P]

            # ---- inputs: chunked fat DMAs ordered so compute can start the
            # moment (wq0, wk0, tm, xt t-chunk 0) land; the rest streams in
            # behind the prelude.  x is host-packed per t-chunk so each chunk
            # stays descriptor-fat (8KB rows).
            # issue in parallel from all three DMA-capable queues — each
            # dma_start costs ~0.8us of sequencer issue time
            xt_r = xt_d.rearrange("p (tc kc t) -> p tc kc t", tc=4, kc=KC)
            nc.sync.dma_start(wpk_sb[:, 0:WV_OFF], wpk_d[:, 0:WV_OFF])
            nc.scalar.dma_start(xt_sb[:, :, 0:512], xt_r[:, 0])
            nc.gpsimd.dma_start(wpk_sb[:, WV_OFF:WQ1_OFF],
                                wpk_d[:, WV_OFF:WQ1_OFF])
            nc.sync.dma_start(xt_sb[:, :, 512:1024], xt_r[:, 1])
            nc.scalar.dma_start(wpk_sb[:, WQ1_OFF:WPK_COLS],
                                wpk_d[:, WQ1_OFF:WPK_COLS])
            nc.gpsimd.dma_start(xt_sb[:, :, 1024:1536], xt_r[:, 2])
            nc.sync.dma_start(xt_sb[:, :, 1536:2048], xt_r[:, 3])
            nc.sync.dma_start(bias_sb[:], bqk_d.rearrange(
                "p (cc r) -> p cc r", cc=2))
            nc.sync.dma_start(bv_row[:], bv_d)

            # early, off the critical path: exp table preload + constants
            nc.vector.memset(warm_sb[:], 0.0)
            nc.scalar.activation(warm_out[:], warm_sb[0:1, 0:8], Exp)
            nc.vector.memset(v_sb[:, :, :, HD:HD + 1], 1.0)
            nc.gpsimd.partition_broadcast(bvb_sb[:], bv_row[:], channels=P)

            # ---- filler groups: QKV projections + output projection --------
            gidx = [0]

            def qk_group(wcc, dst, cc, tb, ridx, use_act=False):
                wsb = wcc[cc]
                def emit():
                    g = gidx[0]; gidx[0] += 1
                    ps = fillps.tile([P, 512], F32, tag=f"fl{g % 2}",
                                     name=f"qk_{g}")
                    for kc in range(KC):
                        nc.tensor.matmul(
                            ps[:],
                            wsb[:, kc, :],
                            xt_sb[:, kc, tb * 512:(tb + 1) * 512],
                            start=(kc == 0), stop=(kc == KC - 1))
                    if use_act:
                        nc.scalar.activation(
                            dst[:, cc, tb * 512:(tb + 1) * 512], ps[:],
                            Ident, bias=bias_sb[:, cc, ridx:ridx + 1])
                    else:
                        nc.vector.tensor_scalar_add(
                            dst[:, cc, tb * 512:(tb + 1) * 512], ps[:],
                            bias_sb[:, cc, ridx:ridx + 1])
                return emit

            def v_group(ji):
                def emit():
                    g = gidx[0]; gidx[0] += 1
                    ps = fillps.tile([P, C], F32, tag=f"fl{g % 2}",
                                     name=f"v_{g}")
                    for kc in range(KC):
                        nc.tensor.matmul(
                            ps[:],
                            xt_sb[:, kc, ji * P:(ji + 1) * P],
                            wv_sb[:, kc, :],
                            start=(kc == 0), stop=(kc == KC - 1))
                    nc.vector.tensor_add(
                        v_sb[:, ji, :, 0:HD],
                        ps[:].rearrange("p (h x) -> p h x", h=NHL),
                        bvb_sb[:].rearrange("p (h x) -> p h x", h=NHL))
                return emit

            def proj_group(t2, ob, use_act=False):
                def emit():
                    g = gidx[0]; gidx[0] += 1
                    pp = fillps.tile([P, 512], F32, tag=f"fl{g % 2}",
                                     name=f"pr_{g}")
                    for cc in range(2):
                        nc.tensor.matmul(
                            pp[:],
                            yt_sb[:, cc, t2 * P:(t2 + 1) * P],
                            wp_sb[:, cc, ob * 512:(ob + 1) * 512],
                            start=(cc == 0), stop=(cc == 1))
                    ot = osb.tile([P, 512], BF, tag="ot", name=f"ot_{g}")
                    # dma_start issue costs ~0.8us of sequencer time, so
                    # spread writebacks over otherwise-idle engine queues:
                    # tail tiles split halves over scalar+gpsimd queues,
                    # steady-state tiles ride the vector queue (in order
                    # behind their own copy).
                    if use_act:
                        nc.scalar.copy(ot[:], pp[:])
                        nc.scalar.dma_start(
                            out_d[t2 * P:t2 * P + 64,
                                  ob * 512:(ob + 1) * 512],
                            ot[0:64, :])
                        nc.gpsimd.dma_start(
                            out_d[t2 * P + 64:(t2 + 1) * P,
                                  ob * 512:(ob + 1) * 512],
                            ot[64:P, :])
                    else:
                        nc.vector.tensor_copy(ot[:], pp[:])
                        nc.gpsimd.dma_start(
                            out_d[t2 * P:(t2 + 1) * P,
                                  ob * 512:(ob + 1) * 512],
                            ot[:])
                return emit

            def dummy_group(n=8):
                # PE no-op padding: keeps the HAM activity window fed across
                # unavoidable dependency stalls (tail normalize chains) so
                # the real matmuls around them stay at full clock.
                def emit():
                    g = gidx[0]; gidx[0] += 1
                    dp = fillps.tile([P, 512], F32, tag=f"fl{g % 2}",
                                     name=f"dm_{g}")
                    for i in range(n):
                        nc.tensor.matmul(dp[:], warm_sb[:, 0:P], warm_sb[:],
                                         start=(i == 0), stop=(i == n - 1))
                return emit

            # HAM pre-warm: dummy matmuls on zeroed scratch keep the PE
            # streaming while the inputs land, so the first real groups run
            # at full clock instead of warming up on real work.
            warm_ps = stps.tile([P, 1024], F32, tag="st", name="warm_ps")
            for w in range(WARM):
                nc.tensor.matmul(warm_ps[:, 0:512], warm_sb[:, 0:P],
                                 warm_sb[:], start=True, stop=True)

            # minimal prelude: only what attention chunk (tb0, pair0, ji=0)
            # needs.  q-copy rides the Scalar engine (same table set as exp)
            # so it overlaps the k-copy on Vector.
            qk_group(wq_cc, qt_sb, 0, 0, 0, use_act=True)()
            qk_group(wk_cc, kt_sb, 0, 0, 1)()
            v_group(0)()

            # ---- per-t-block filler schedules ------------------------------
            # stage-qk(tb) must land before tb starts; v(ji) before the
            # att@V that consumes it; proj(t2) is deferred into the
            # exp-heavy late t-blocks so the tail PE stream stays dense.
            fillers_by_tb = [
                # tb0: rest of stage0 (pair1 q/k, v1-3) then all of stage1 qk
                [v_group(1), v_group(2),
                 qk_group(wq_cc, qt_sb, 1, 0, 0),
                 v_group(3),
                 qk_group(wk_cc, kt_sb, 1, 0, 1),
                 qk_group(wq_cc, qt_sb, 0, 1, 0),
                 qk_group(wk_cc, kt_sb, 0, 1, 1),
                 qk_group(wq_cc, qt_sb, 1, 1, 0),
                 qk_group(wk_cc, kt_sb, 1, 1, 1)],
                # tb1: stage1 v, stage2 qk
                [v_group(4), v_group(5), v_group(6), v_group(7),
                 qk_group(wq_cc, qt_sb, 0, 2, 0),
                 qk_group(wk_cc, kt_sb, 0, 2, 1),
                 qk_group(wq_cc, qt_sb, 1, 2, 0),
                 qk_group(wk_cc, kt_sb, 1, 2, 1)],
                # tb2: stage2 v, stage3 qk, first proj batch
                [v_group(8), v_group(9), v_group(10), v_group(11),
                 qk_group(wq_cc, qt_sb, 0, 3, 0),
                 qk_group(wk_cc, kt_sb, 0, 3, 1),
                 qk_group(wq_cc, qt_sb, 1, 3, 0),
                 qk_group(wk_cc, kt_sb, 1, 3, 1)]
                + [proj_group(t2, ob) for t2 in range(0, 4)
                   for ob in range(2)],
                # tb3: stage3 v early (needed by chunks 12-15), then proj,
                # then HAM-padding for the chase region
                [v_group(12), v_group(13), v_group(14), v_group(15)]
                + [proj_group(t2, ob) for t2 in range(4, 12)
                   for ob in range(2)]
                + [dummy_group(8), dummy_group(8)],
            ]

            # ---- attention: LAG-pipelined, filler-padded -------------------
            for tb in range(4):
                tsl = slice(tb * 512, (tb + 1) * 512)
                fillers = fillers_by_tb[tb]
                njc = 4 * tb + 4
                total_iters = 2 * njc
                it = [0]
                done = [0]

                def drip():
                    while done[0] * total_iters < len(fillers) * it[0]:
                        fillers[done[0]]()
                        done[0] += 1

                for hp in (0, 2):
                    pair = (hp, hp + 1)
                    final_pair = (tb == 3 and hp == 2)
                    lag = 1 if final_pair else LAG
                    yps = {h: ytps.tile([P, 512], F32, tag="yt",
                                        name=f"yt_{tb}_{h}")
                           for h in pair}

                    def mm1_pair(ji, _tb=tb, _hp=hp, _pair=pair):
                        off = max(0, (ji - 4 * _tb) * P)
                        stp = stps.tile([P, 1024], F32, tag="st",
                                        name=f"st_{_tb}_{_hp}_{ji}")
                        for h in _pair:
                            bse = 64 * (h % 2)
                            cc = h // 2
                            nc.tensor.matmul(
                                stp[:, 512 * (h - _hp) + off:512 * (h - _hp) + 512],
                                kt_sb[bse:bse + 64, cc, ji * P:(ji + 1) * P],
                                qt_sb[bse:bse + 64, cc,
                                      _tb * 512 + off:(_tb + 1) * 512],
                                start=True, stop=True)
                        return stp, off

                    def att_av(ji, et, off, _tb=tb, _hp=hp, _pair=pair,
                               _njc=njc, _final=final_pair, _yps=yps):
                        for h in _pair:
                            nc.tensor.matmul(
                                _yps[h][:HD + 1, off:],
                                v_sb[:, ji, h, :],
                                et[:, 512 * (h - _hp) + off:512 * (h - _hp) + 512],
                                start=(ji == 0),
                                stop=(True if _final else ji == _njc - 1),
                                skip_group_check=True)
                        if _final and ji >= _njc - 4:
                            # columns < 128*(ji-11) are final: normalize the
                            # quarter and chase with its output projection to
                            # keep the kernel tail short and PE-dense
                            qtr = ji - (_njc - 4)
                            qs = slice(qtr * P, (qtr + 1) * P)
                            for h in _pair:
                                bse = 64 * (h % 2)
                                # custom-DVE ops break at partition base != 0,
                                # so: aligned copy out of PSUM, gpsimd
                                # broadcast of the denominator row down to
                                # base 0, approx-reciprocal there, multiply.
                                yaq = nrm.tile([HD + 1, P], F32, tag="yaq",
                                               name=f"yaq_{qtr}_{h}")
                                nc.vector.tensor_copy(yaq[:],
                                                      _yps[h][:HD + 1, qs])
                                rrq = nrm.tile([1, P], F32, tag="rrq",
                                               name=f"rrq_{qtr}_{h}")
                                nc.sync.dma_start(rrq[:], yaq[HD:HD + 1, :])
                                recq = nrm.tile([1, P], F32, tag="recq",
                                                name=f"recq_{qtr}_{h}")
                                nc.vector.reciprocal_approx_fast(recq[:],
                                                                 rrq[:])
                                bcq = nrm.tile([HD, P], F32, tag="bcq",
                                               name=f"bcq_{qtr}_{h}")
                                nc.gpsimd.partition_broadcast(bcq[:], recq[:],
                                                              channels=HD)
                                nc.vector.tensor_mul(
                                    yt_sb[bse:bse + 64, 1,
                                          1536 + qtr * P:1536 + (qtr + 1) * P],
                                    yaq[0:HD, :], bcq[:])
                            # pad the PE across the normalize-chain latency
                            # so HAM stays at full clock into the projections
                            dummy_group(8)()
                            for ob in range(2):
                                proj_group(12 + qtr, ob, use_act=True)()

                    pend = []
                    for ji in range(njc):
                        stp, off = mm1_pair(ji)
                        et = esb.tile([P, 1024], BF, tag="et",
                                      name=f"et_{tb}_{hp}_{ji}")
                        stp3 = stp[:].rearrange("p (g c) -> p g c", g=2)
                        et3 = et[:].rearrange("p (g c) -> p g c", g=2)
                        nc.scalar.activation(et3[:, :, off:], stp3[:, :, off:],
                                             Exp, scale=0.125)
                        if ji >= 4 * tb:
                            nc.vector.tensor_mul(
                                et3[:, :, off:off + P],
                                et3[:, :, off:off + P],
                                tm_sb[:, None, :].to_broadcast([P, 2, P]))
                        pend.append((ji, et, off))
                        it[0] += 1
                        drip()
                        if len(pend) > lag:
                            att_av(*pend.pop(0))
                    while pend:
                        drip()
                        att_av(*pend.pop(0))

                    if final_pair:
                        continue  # fast tail path handled in att_av
                    # normalization: reciprocal straight off the PSUM
                    # denominator row (row 64), broadcast across partitions
                    # on the idle GpSimd engine, one multiply out.
                    for h in pair:
                        bse = 64 * (h % 2)
                        cc = h // 2
                        # aligned PSUM evacuation; one SBUF->SBUF DMA hop
                        # moves the denominator row to partition base 0
                        # (custom-DVE recip and partition_broadcast both
                        # misread APs with partition base != 0).
                        ya = nrm.tile([HD + 1, 512], F32, tag="ya",
                                      name=f"ya_{tb}_{h}")
                        nc.vector.tensor_copy(ya[:], yps[h][:HD + 1, :])
                        rr = nrm.tile([1, 512], F32, tag="rr",
                                      name=f"rr_{tb}_{h}")
                        nc.sync.dma_start(rr[:], ya[HD:HD + 1, :])
                        rbc = nrm.tile([1, 512], F32, tag="rbc",
                                       name=f"rbc_{tb}_{h}")
                        nc.vector.reciprocal_approx_fast(rbc[:], rr[:])
                        bc = nrm.tile([HD, 512], F32, tag="bc",
                                      name=f"bc_{tb}_{h}")
                        nc.gpsimd.partition_broadcast(bc[:], rbc[:],
                                                      channels=HD)
                        if DEBUG:
                            if not hasattr(nc, "_dbg_rec"):
                                nc._dbg_rec = sing.tile([16, 512], F32,
                                                        name="dbg_rec")
                                nc._dbg_den = sing.tile([16, 512], F32,
                                                        name="dbg_den")
                            k = tb * 4 + h
                            nc.sync.dma_start(nc._dbg_rec[k:k + 1, :],
                                              rbc[0:1, :])
                            nc.sync.dma_start(nc._dbg_den[k:k + 1, :],
                                              ya[HD:HD + 1, :])
                            if tb == 1 and h == 1:
                                nc._dbg_yu = sing.tile([HD, 512], F32,
                                                       name="dbg_yu")
                                nc.vector.tensor_copy(nc._dbg_yu[:],
                                                      ya[0:HD, :])
                        nc.vector.tensor_mul(yt_sb[bse:bse + 64, cc, tsl],
                                             ya[0:HD, :], bc[:])
                while done[0] < len(fillers):
                    fillers[done[0]]()
                    done[0] += 1

            if DEBUG:
                nc.sync.dma_start(qt_o, qt_sb[:].rearrange("p a b -> p (a b)"))
                nc.sync.dma_start(kt_o, kt_sb[:].rearrange("p a b -> p (a b)"))
                nc.sync.dma_start(v_o, v_sb[:].rearrange("p a b c -> p (a b c)"))
                nc.sync.dma_start(yt_o, yt_sb[:].rearrange("p a b -> p (a b)"))
                nc.sync.dma_start(rec_o, nc._dbg_rec[:])
                nc.sync.dma_start(den_o, nc._dbg_den[:])
                nc.sync.dma_start(yu_o, nc._dbg_yu[:])

    nc.compile()
    return nc


_NC = None


def _get_nc():
    global _NC
    if _NC is None:
        _NC = build_graph()
    return _NC


def make_in_maps(x, Wq, bq, Wk, bk, Wv, bv, Wp, bp):
    x = np.asarray(x, np.float32)
    tm = np.triu(np.ones((P, P), np.float32))  # keep where p <= f
    in_maps = []
    for core in range(N_CORES):
        b = core // 4
        hg = core % 4
        rs = slice(hg * C, (hg + 1) * C)
        xt = np.ascontiguousarray(x[b].T)  # [DP, T]
        # [p, tc, kc, t'] so each 512-wide t-chunk is one descriptor-fat DMA
        xpk = xt.reshape(KC, P, 4, 512).transpose(1, 2, 0, 3).reshape(
            P, KC * T)
        m = {"xt": xpk.astype(bf16)}
        wpk = np.empty((P, WPK_COLS), np.float32)
        for (o0, o1), W in (((WQ0_OFF, WQ1_OFF), Wq),
                            ((WK0_OFF, WK1_OFF), Wk)):
            ws = np.ascontiguousarray(np.asarray(W, np.float32)[rs].T)
            wr = ws.reshape(KC, P, 2, P)       # [kc, p, cc, c]
            wpk[:, o0:o0 + 1024] = wr[:, :, 0].transpose(1, 0, 2).reshape(
                P, KC * P)
            wpk[:, o1:o1 + 1024] = wr[:, :, 1].transpose(1, 0, 2).reshape(
                P, KC * P)
        ws = np.ascontiguousarray(np.asarray(Wv, np.float32)[rs].T)
        wpk[:, WV_OFF:WV_OFF + 2048] = ws.reshape(KC, P, C).transpose(
            1, 0, 2).reshape(P, KC * C)
        wps = np.ascontiguousarray(np.asarray(Wp, np.float32)[:, rs].T)
        wpk[:, WP_OFF:WP_OFF + 2048] = wps.reshape(2, P, D).transpose(
            1, 0, 2).reshape(P, 2 * D)
        wpk[:, TM_OFF:TM_OFF + P] = tm
        m["wpk"] = wpk.astype(bf16)
        bqs = np.asarray(bq, np.float32)[rs].reshape(2, P)
        bks = np.asarray(bk, np.float32)[rs].reshape(2, P)
        m["bqk"] = np.stack([bqs[0], bks[0], bqs[1], bks[1]], axis=1)
        m["bv"] = np.asarray(bv, np.float32)[rs].reshape(1, C)
        in_maps.append(m)
    return in_maps


def kernel(x, Wq, bq, Wk, bk, Wv, bv, Wp, bp, _trace=False):
    nc = _get_nc()
    in_maps = make_in_maps(x, Wq, bq, Wk, bk, Wv, bv, Wp, bp)
    res = bass_utils.run_bass_kernel_spmd(
        nc, in_maps, core_ids=list(range(N_CORES)), trace=_trace)
    kernel.last_exec_time_ns = res.exec_time_ns
    bp = np.asarray(bp, np.float32)
    out = np.empty((B, T, D), np.float32)
    for b in range(B):
        acc = np.zeros((T, D), np.float32)
        for hg in range(4):
            acc += res.results[4 * b + hg]["out"].astype(np.float32)
        out[b] = acc + bp
    return out


# revision 32
# speedup vs baseline: 1.0587x; 1.0587x over previous
"""Causal self-attention (B=2, T=2048, D=1024, H=16) on 8 Trainium2 NeuronCores.

Sharding: data-parallel on batch (2-way) x tensor-parallel on heads (4-way):
each core owns one batch's activations and 4 heads (256 channels) of the
QKV / output-projection weights.  Host pre-transposes x and packs all weight
shards into one [128, 8320] tensor laid out exactly as SBUF wants them, so
the whole input loads in ~3 DMAs with 32KB/16KB-per-partition descriptors:
  qT/kT = W[heads] @ x.T        (channels on partitions, T on free axis)
  v     = x @ Wv[heads].T       (T on partitions) + ones column (softmax sum)
  expST[j,t] = exp(0.125 * k_h q_h^T)   (kv-position on partitions)
  yT_aug = v_aug.T @ expST      (row 64 = softmax denominator)
  yT     = yT_aug[:64] * recip(denom) broadcast across partitions
  partial out = yT.T @ WpT[heads]  -> [T, D] partial per core, summed on host.
Causality: fully-masked 128-col j-blocks are skipped, partially-masked columns
sliced away, and one 128x128 triangular mask multiplies the diagonal block.
Softmax skips max-subtraction (scores are O(1) by construction).
The two heads of a pair sit at partition bases 0/64 so their K=64 score
matmuls run concurrently in separate PE row-groups.

Pipeline discipline (the point of this version): the PE must stream
back-to-back or the HAM clock-gate drops it to half clock.  So
  - attention starts as early as the first q/k/v tiles exist (~10us, not
    ~40us): the prelude computes only (q,k) for head-pair 0 of t-block 0
    plus v for the first kv block; the rest of QKV is dripped as filler,
  - att@V consumption LAGS the exp stream by 2 chunks so the PE never
    blocks on the Scalar engine; filler groups are dripped BEFORE each
    dependent att@V,
  - softmax normalization uses reciprocal_approx_fast straight off the
    PSUM denominator row + a gpsimd partition-broadcast (no DMA
    round-trips on the critical path),
  - the output projection is deferred into the exp-heavy late t-blocks and
    the final head-pair chases quarter-blocks so the tail stays dense.
"""
import sys, types

for _p in ("/opt/trn_rl_repo",):
    if _p not in sys.path:
        sys.path.append(_p)


def _install_ntff_hook():
    """Register the axon NTFF profile hook that container boot skips when
    antenv.axon_hooks is absent (needed only for profiled runs)."""
    if "antenv.axon_hooks" in sys.modules:
        return
    mod = types.ModuleType("antenv.axon_hooks")
    _h = [None]
    mod.set_axon_ntff_profile_hook = lambda h: _h.__setitem__(0, h)
    mod.get_axon_ntff_profile_hook = lambda: _h[0]
    sys.modules["antenv.axon_hooks"] = mod
    try:
        import antenv
        antenv.axon_hooks = mod
    except Exception:
        pass
    try:
        from trn_agent_boot.trn_boot import _ntff_profile_via_ctypes
        mod.set_axon_ntff_profile_hook(
            _ntff_profile_via_ctypes("/opt/axon/libaxon_pjrt.so"))
    except Exception:
        pass


_install_ntff_hook()

import numpy as np
import ml_dtypes

import concourse.tile as tile
from concourse import bacc, mybir, bass_utils

B, T, D, H = 2, 2048, 1024, 16
HD = 64
NHL = 4            # heads per core
C = NHL * HD       # 256 channels per core
DP = 1024          # contraction dim (biases added separately)
KC = DP // 128     # 8
P = 128
BF = mybir.dt.bfloat16
F32 = mybir.dt.float32
bf16 = ml_dtypes.bfloat16

N_CORES = 8
WARM = 16          # HAM pre-warm matmuls covering the input-DMA window
LAG = 2            # chunks the att@V stream trails the exp stream by
DEBUG = False      # add qt/kt/v/yt DRAM taps for numeric debugging

# packed weight layout offsets (bf16 columns per partition), ordered so the
# first DMA chunk carries exactly what attention chunk 0 needs
WQ0_OFF, WK0_OFF, TM_OFF = 0, 1024, 2048
WV_OFF, WQ1_OFF, WK1_OFF, WP_OFF = 2176, 4224, 5248, 6272
WPK_COLS = 8320


def build_graph():
    nc = bacc.Bacc("TRN2", target_bir_lowering=False, debug=False,
                   num_devices=N_CORES)
    xt_d = nc.dram_tensor("xt", [P, KC * T], BF, kind="ExternalInput").ap()
    wpk_d = nc.dram_tensor("wpk", [P, WPK_COLS], BF, kind="ExternalInput").ap()
    bqk_d = nc.dram_tensor("bqk", [P, 4], F32, kind="ExternalInput").ap()
    bv_d = nc.dram_tensor("bv", [1, C], F32, kind="ExternalInput").ap()
    out_d = nc.dram_tensor("out", [T, D], BF, kind="ExternalOutput").ap()
    if DEBUG:
        qt_o = nc.dram_tensor("qt_o", [P, 2 * T], BF, kind="ExternalOutput").ap()
        kt_o = nc.dram_tensor("kt_o", [P, 2 * T], BF, kind="ExternalOutput").ap()
        v_o = nc.dram_tensor("v_o", [P, 16 * NHL * (HD + 1)], BF,
                             kind="ExternalOutput").ap()
        yt_o = nc.dram_tensor("yt_o", [P, 2 * T], BF, kind="ExternalOutput").ap()
        rec_o = nc.dram_tensor("rec_o", [16, 512], F32, kind="ExternalOutput").ap()
        den_o = nc.dram_tensor("den_o", [16, 512], F32, kind="ExternalOutput").ap()
        yu_o = nc.dram_tensor("yu_o", [HD, 512], F32, kind="ExternalOutput").ap()

    Exp = mybir.ActivationFunctionType.Exp
    Ident = mybir.ActivationFunctionType.Identity

    with tile.TileContext(nc) as tc:
        with tc.tile_pool(name="sing", bufs=1) as sing, \
             tc.tile_pool(name="fill", bufs=1, space="PSUM") as fillps, \
             tc.tile_pool(name="stps", bufs=2, space="PSUM") as stps, \
             tc.tile_pool(name="ytps", bufs=2, space="PSUM") as ytps, \
             tc.tile_pool(name="esb", bufs=4) as esb, \
             tc.tile_pool(name="nrm", bufs=6) as nrm, \
             tc.tile_pool(name="osb", bufs=4) as osb:
            xt_sb = sing.tile([P, KC, T], BF)
            wpk_sb = sing.tile([P, WPK_COLS], BF)
            qt_sb = sing.tile([P, 2, T], BF)
            kt_sb = sing.tile([P, 2, T], BF)
            v_sb = sing.tile([P, 16, NHL, HD + 1], BF)
            yt_sb = sing.tile([P, 2, T], BF)
            bias_sb = sing.tile([P, 2, 2], F32)
            bv_row = sing.tile([1, C], F32)
            bvb_sb = sing.tile([P, C], F32)
            warm_sb = sing.tile([P, 512], BF)
            warm_out = sing.tile([1, 8], BF)

            wq_cc = [wpk_sb[:, WQ0_OFF:WQ0_OFF + 1024].rearrange(
                         "p (kc c) -> p kc c", kc=KC),
                     wpk_sb[:, WQ1_OFF:WQ1_OFF + 1024].rearrange(
                         "p (kc c) -> p kc c", kc=KC)]
            wk_cc = [wpk_sb[:, WK0_OFF:WK0_OFF + 1024].rearrange(
                         "p (kc c) -> p kc c", kc=KC),
                     wpk_sb[:, WK1_OFF:WK1_OFF + 1024].rearrange(
                         "p (kc c) -> p kc c", kc=KC)]
            wv_sb = wpk_sb[:, WV_OFF:WV_OFF + 2048].rearrange(
                "p (kc c) -> p kc c", kc=KC)
            wp_sb = wpk_sb[:, WP_OFF:WP_OFF + 2048].rearrange(
                "p (cc o) -> p cc o", cc=2)
            tm_sb = wpk_sb[:, TM_OFF:TM_OFF + P]

            # ---- inputs: chunked fat DMAs ordered so compute can start the
            # moment (wq0, wk0, tm, xt t-chunk 0) land; the rest streams in
            # behind the prelude.  x is host-packed per t-chunk so each chunk
            # stays descriptor-fat (8KB rows).
            xt_r = xt_d.rearrange("p (tc kc t) -> p tc kc t", tc=4, kc=KC)
            nc.sync.dma_start(wpk_sb[:, 0:WV_OFF], wpk_d[:, 0:WV_OFF])
            nc.sync.dma_start(bias_sb[:], bqk_d.rearrange(
                "p (cc r) -> p cc r", cc=2))
            nc.sync.dma_start(bv_row[:], bv_d)
            nc.sync.dma_start(xt_sb[:, :, 0:512], xt_r[:, 0])
            nc.sync.dma_start(wpk_sb[:, WV_OFF:WQ1_OFF],
                              wpk_d[:, WV_OFF:WQ1_OFF])
            nc.sync.dma_start(xt_sb[:, :, 512:1024], xt_r[:, 1])
            nc.sync.dma_start(wpk_sb[:, WQ1_OFF:WPK_COLS],
                              wpk_d[:, WQ1_OFF:WPK_COLS])
            nc.sync.dma_start(xt_sb[:, :, 1024:1536], xt_r[:, 2])
            nc.sync.dma_start(xt_sb[:, :, 1536:2048], xt_r[:, 3])

            # early, off the critical path: exp table preload + constants
            nc.vector.memset(warm_sb[:], 0.0)
            nc.scalar.activation(warm_out[:], warm_sb[0:1, 0:8], Exp)
            nc.vector.memset(v_sb[:, :, :, HD:HD + 1], 1.0)
            nc.gpsimd.partition_broadcast(bvb_sb[:], bv_row[:], channels=P)

            # ---- filler groups: QKV projections + output projection --------
            gidx = [0]

            def qk_group(wcc, dst, cc, tb, ridx, use_act=False):
                wsb = wcc[cc]
                def emit():
                    g = gidx[0]; gidx[0] += 1
                    ps = fillps.tile([P, 512], F32, tag=f"fl{g % 2}",
                                     name=f"qk_{g}")
                    for kc in range(KC):
                        nc.tensor.matmul(
                            ps[:],
                            wsb[:, kc, :],
                            xt_sb[:, kc, tb * 512:(tb + 1) * 512],
                            start=(kc == 0), stop=(kc == KC - 1))
                    if use_act:
                        nc.scalar.activation(
                            dst[:, cc, tb * 512:(tb + 1) * 512], ps[:],
                            Ident, bias=bias_sb[:, cc, ridx:ridx + 1])
                    else:
                        nc.vector.tensor_scalar_add(
                            dst[:, cc, tb * 512:(tb + 1) * 512], ps[:],
                            bias_sb[:, cc, ridx:ridx + 1])
                return emit

            def v_group(ji):
                def emit():
                    g = gidx[0]; gidx[0] += 1
                    ps = fillps.tile([P, C], F32, tag=f"fl{g % 2}",
                                     name=f"v_{g}")
                    for kc in range(KC):
                        nc.tensor.matmul(
                            ps[:],
                            xt_sb[:, kc, ji * P:(ji + 1) * P],
                            wv_sb[:, kc, :],
                            start=(kc == 0), stop=(kc == KC - 1))
                    nc.vector.tensor_add(
                        v_sb[:, ji, :, 0:HD],
                        ps[:].rearrange("p (h x) -> p h x", h=NHL),
                        bvb_sb[:].rearrange("p (h x) -> p h x", h=NHL))
                return emit

            def proj_group(t2, ob, use_act=False):
                def emit():
                    g = gidx[0]; gidx[0] += 1
                    pp = fillps.tile([P, 512], F32, tag=f"fl{g % 2}",
                                     name=f"pr_{g}")
                    for cc in range(2):
                        nc.tensor.matmul(
                            pp[:],
                            yt_sb[:, cc, t2 * P:(t2 + 1) * P],
                            wp_sb[:, cc, ob * 512:(ob + 1) * 512],
                            start=(cc == 0), stop=(cc == 1))
                    ot = osb.tile([P, 512], BF, tag="ot", name=f"ot_{g}")
                    # dma_start issue costs ~0.8us of sequencer time, so
                    # spread writebacks over otherwise-idle engine queues:
                    # tail tiles split halves over scalar+gpsimd queues,
                    # steady-state tiles ride the vector queue (in order
                    # behind their own copy).
                    if use_act:
                        nc.scalar.copy(ot[:], pp[:])
                        nc.scalar.dma_start(
                            out_d[t2 * P:t2 * P + 64,
                                  ob * 512:(ob + 1) * 512],
                            ot[0:64, :])
                        nc.gpsimd.dma_start(
                            out_d[t2 * P + 64:(t2 + 1) * P,
                                  ob * 512:(ob + 1) * 512],
                            ot[64:P, :])
                    else:
                        nc.vector.tensor_copy(ot[:], pp[:])
                        nc.gpsimd.dma_start(
                            out_d[t2 * P:(t2 + 1) * P,
                                  ob * 512:(ob + 1) * 512],
                            ot[:])
                return emit

            def dummy_group(n=8):
                # PE no-op padding: keeps the HAM activity window fed across
                # unavoidable dependency stalls (tail normalize chains) so
                # the real matmuls around them stay at full clock.
                def emit():
                    g = gidx[0]; gidx[0] += 1
                    dp = fillps.tile([P, 512], F32, tag=f"fl{g % 2}",
                                     name=f"dm_{g}")
                    for i in range(n):
                        nc.tensor.matmul(dp[:], warm_sb[:, 0:P], warm_sb[:],
                                         start=(i == 0), stop=(i == n - 1))
                return emit

            # HAM pre-warm: dummy matmuls on zeroed scratch keep the PE
            # streaming while the inputs land, so the first real groups run
            # at full clock instead of warming up on real work.
            warm_ps = stps.tile([P, 1024], F32, tag="st", name="warm_ps")
            for w in range(WARM):
                nc.tensor.matmul(warm_ps[:, 0:512], warm_sb[:, 0:P],
                                 warm_sb[:], start=True, stop=True)

            # minimal prelude: only what attention chunk (tb0, pair0, ji=0)
            # needs.  q-copy rides the Scalar engine (same table set as exp)
            # so it overlaps the k-copy on Vector.
            qk_group(wq_cc, qt_sb, 0, 0, 0, use_act=True)()
            qk_group(wk_cc, kt_sb, 0, 0, 1)()
            v_group(0)()

            # ---- per-t-block filler schedules ------------------------------
            # stage-qk(tb) must land before tb starts; v(ji) before the
            # att@V that consumes it; proj(t2) is deferred into the
            # exp-heavy late t-blocks so the tail PE stream stays dense.
            fillers_by_tb = [
                # tb0: rest of stage0 (pair1 q/k, v1-3) then all of stage1 qk
                [v_group(1), v_group(2),
                 qk_group(wq_cc, qt_sb, 1, 0, 0),
                 v_group(3),
                 qk_group(wk_cc, kt_sb, 1, 0, 1),
                 qk_group(wq_cc, qt_sb, 0, 1, 0),
                 qk_group(wk_cc, kt_sb, 0, 1, 1),
                 qk_group(wq_cc, qt_sb, 1, 1, 0),
                 qk_group(wk_cc, kt_sb, 1, 1, 1)],
                # tb1: stage1 v, stage2 qk
                [v_group(4), v_group(5), v_group(6), v_group(7),
                 qk_group(wq_cc, qt_sb, 0, 2, 0),
                 qk_group(wk_cc, kt_sb, 0, 2, 1),
                 qk_group(wq_cc, qt_sb, 1, 2, 0),
                 qk_group(wk_cc, kt_sb, 1, 2, 1)],
                # tb2: stage2 v, stage3 qk, first proj batch
                [v_group(8), v_group(9), v_group(10), v_group(11),
                 qk_group(wq_cc, qt_sb, 0, 3, 0),
                 qk_group(wk_cc, kt_sb, 0, 3, 1),
                 qk_group(wq_cc, qt_sb, 1, 3, 0),
                 qk_group(wk_cc, kt_sb, 1, 3, 1)]
                + [proj_group(t2, ob) for t2 in range(0, 4)
                   for ob in range(2)],
                # tb3: stage3 v early (needed by chunks 12-15), then proj,
                # then HAM-padding for the chase region
                [v_group(12), v_group(13), v_group(14), v_group(15)]
                + [proj_group(t2, ob) for t2 in range(4, 12)
                   for ob in range(2)]
                + [dummy_group(8), dummy_group(8)],
            ]

            # ---- attention: LAG-pipelined, filler-padded -------------------
            for tb in range(4):
                tsl = slice(tb * 512, (tb + 1) * 512)
                fillers = fillers_by_tb[tb]
                njc = 4 * tb + 4
                total_iters = 2 * njc
                it = [0]
                done = [0]

                def drip():
                    while done[0] * total_iters < len(fillers) * it[0]:
                        fillers[done[0]]()
                        done[0] += 1

                for hp in (0, 2):
                    pair = (hp, hp + 1)
                    final_pair = (tb == 3 and hp == 2)
                    lag = 1 if final_pair else LAG
                    yps = {h: ytps.tile([P, 512], F32, tag="yt",
                                        name=f"yt_{tb}_{h}")
                           for h in pair}

                    def mm1_pair(ji, _tb=tb, _hp=hp, _pair=pair):
                        off = max(0, (ji - 4 * _tb) * P)
                        stp = stps.tile([P, 1024], F32, tag="st",
                                        name=f"st_{_tb}_{_hp}_{ji}")
                        for h in _pair:
                            bse = 64 * (h % 2)
                            cc = h // 2
                            nc.tensor.matmul(
                                stp[:, 512 * (h - _hp) + off:512 * (h - _hp) + 512],
                                kt_sb[bse:bse + 64, cc, ji * P:(ji + 1) * P],
                                qt_sb[bse:bse + 64, cc,
                                      _tb * 512 + off:(_tb + 1) * 512],
                                start=True, stop=True)
                        return stp, off

                    def att_av(ji, et, off, _tb=tb, _hp=hp, _pair=pair,
                               _njc=njc, _final=final_pair, _yps=yps):
                        for h in _pair:
                            nc.tensor.matmul(
                                _yps[h][:HD + 1, off:],
                                v_sb[:, ji, h, :],
                                et[:, 512 * (h - _hp) + off:512 * (h - _hp) + 512],
                                start=(ji == 0),
                                stop=(True if _final else ji == _njc - 1),
                                skip_group_check=True)
                        if _final and ji >= _njc - 4:
                            # columns < 128*(ji-11) are final: normalize the
                            # quarter and chase with its output projection to
                            # keep the kernel tail short and PE-dense
                            qtr = ji - (_njc - 4)
                            qs = slice(qtr * P, (qtr + 1) * P)
                            for h in _pair:
                                bse = 64 * (h % 2)
                                # custom-DVE ops break at partition base != 0,
                                # so: aligned copy out of PSUM, gpsimd
                                # broadcast of the denominator row down to
                                # base 0, approx-reciprocal there, multiply.
                                yaq = nrm.tile([HD + 1, P], F32, tag="yaq",
                                               name=f"yaq_{qtr}_{h}")
                                nc.vector.tensor_copy(yaq[:],
                                                      _yps[h][:HD + 1, qs])
                                rrq = nrm.tile([1, P], F32, tag="rrq",
                                               name=f"rrq_{qtr}_{h}")
                                nc.sync.dma_start(rrq[:], yaq[HD:HD + 1, :])
                                recq = nrm.tile([1, P], F32, tag="recq",
                                                name=f"recq_{qtr}_{h}")
                                nc.vector.reciprocal_approx_fast(recq[:],
                                                                 rrq[:])
                                bcq = nrm.tile([HD, P], F32, tag="bcq",
                                               name=f"bcq_{qtr}_{h}")
                                nc.gpsimd.partition_broadcast(bcq[:], recq[:],
                                                              channels=HD)
                                nc.vector.tensor_mul(
                                    yt_sb[bse:bse + 64, 1,
                                          1536 + qtr * P:1536 + (qtr + 1) * P],
                                    yaq[0:HD, :], bcq[:])
                            # pad the PE across the normalize-chain latency
                            # so HAM stays at full clock into the projections
                            dummy_group(8)()
                            for ob in range(2):
                                proj_group(12 + qtr, ob, use_act=True)()

                    pend = []
                    for ji in range(njc):
                        stp, off = mm1_pair(ji)
                        et = esb.tile([P, 1024], BF, tag="et",
                                      name=f"et_{tb}_{hp}_{ji}")
                        stp3 = stp[:].rearrange("p (g c) -> p g c", g=2)
                        et3 = et[:].rearrange("p (g c) -> p g c", g=2)
                        nc.scalar.activation(et3[:, :, off:], stp3[:, :, off:],
                                             Exp, scale=0.125)
                        if ji >= 4 * tb:
                            nc.vector.tensor_mul(
                                et3[:, :, off:off + P],
                                et3[:, :, off:off + P],
                                tm_sb[:, None, :].to_broadcast([P, 2, P]))
                        pend.append((ji, et, off))
                        it[0] += 1
                        drip()
                        if len(pend) > lag:
                            att_av(*pend.pop(0))
                    while pend:
                        drip()
                        att_av(*pend.pop(0))

                    if final_pair:
                        continue  # fast tail path handled in att_av
                    # normalization: reciprocal straight off the PSUM
                    # denominator row (row 64), broadcast across partitions
                    # on the idle GpSimd engine, one multiply out.
                    for h in pair:
                        bse = 64 * (h % 2)
                        cc = h // 2
                        # aligned PSUM evacuation; one SBUF->SBUF DMA hop
                        # moves the denominator row to partition base 0
                        # (custom-DVE recip and partition_broadcast both
                        # misread APs with partition base != 0).
                        ya = nrm.tile([HD + 1, 512], F32, tag="ya",
                                      name=f"ya_{tb}_{h}")
                        nc.vector.tensor_copy(ya[:], yps[h][:HD + 1, :])
                        rr = nrm.tile([1, 512], F32, tag="rr",
                                      name=f"rr_{tb}_{h}")
                        nc.sync.dma_start(rr[:], ya[HD:HD + 1, :])
                        rbc = nrm.tile([1, 512], F32, tag="rbc",
                                       name=f"rbc_{tb}_{h}")
                        nc.vector.reciprocal_approx_fast(rbc[:], rr[:])
                        bc = nrm.tile([HD, 512], F32, tag="bc",
                                      name=f"bc_{tb}_{h}")
                        nc.gpsimd.partition_broadcast(bc[:], rbc[:],
                                                      channels=HD)
                        if DEBUG:
                            if not hasattr(nc, "_dbg_rec"):
                                nc._dbg_rec = sing.tile([16, 512], F32,
                                                        name="dbg_rec")
                                nc._dbg_den = sing.tile([16, 512], F32,
                                                        name="dbg_den")
                            k = tb * 4 + h
                            nc.sync.dma_start(nc._dbg_rec[k:k + 1, :],
                                              rbc[0:1, :])
                            nc.sync.dma_start(nc._dbg_den[k:k + 1, :],
                                              ya[HD:HD + 1, :])
                            if tb == 1 and h == 1:
                                nc._dbg_yu = sing.tile([HD, 512], F32,
                                                       name="dbg_yu")
                                nc.vector.tensor_copy(nc._dbg_yu[:],
                                                      ya[0:HD, :])
                        nc.vector.tensor_mul(yt_sb[bse:bse + 64, cc, tsl],
                                             ya[0:HD, :], bc[:])
                while done[0] < len(fillers):
                    fillers[done[0]]()
                    done[0] += 1

            if DEBUG:
                nc.sync.dma_start(qt_o, qt_sb[:].rearrange("p a b -> p (a b)"))
                nc.sync.dma_start(kt_o, kt_sb[:].rearrange("p a b -> p (a b)"))
                nc.sync.dma_start(v_o, v_sb[:].rearrange("p a b c -> p (a b c)"))
                nc.sync.dma_start(yt_o, yt_sb[:].rearrange("p a b -> p (a b)"))
                nc.sync.dma_start(rec_o, nc._dbg_rec[:])
                nc.sync.dma_start(den_o, nc._dbg_den[:])
                nc.sync.dma_start(yu_o, nc._dbg_yu[:])

    nc.compile()
    return nc


_NC = None


def _get_nc():
    global _NC
    if _NC is None:
        _NC = build_graph()
    return _NC


def make_in_maps(x, Wq, bq, Wk, bk, Wv, bv, Wp, bp):
    x = np.asarray(x, np.float32)
    tm = np.triu(np.ones((P, P), np.float32))  # keep where p <= f
    in_maps = []
    for core in range(N_CORES):
        b = core // 4
        hg = core % 4
        rs = slice(hg * C, (hg + 1) * C)
        xt = np.ascontiguousarray(x[b].T)  # [DP, T]
        # [p, tc, kc, t'] so each 512-wide t-chunk is one descriptor-fat DMA
        xpk = xt.reshape(KC, P, 4, 512).transpose(1, 2, 0, 3).reshape(
            P, KC * T)
        m = {"xt": xpk.astype(bf16)}
        wpk = np.empty((P, WPK_COLS), np.float32)
        for (o0, o1), W in (((WQ0_OFF, WQ1_OFF), Wq),
                            ((WK0_OFF, WK1_OFF), Wk)):
            ws = np.ascontiguousarray(np.asarray(W, np.float32)[rs].T)
            wr = ws.reshape(KC, P, 2, P)       # [kc, p, cc, c]
            wpk[:, o0:o0 + 1024] = wr[:, :, 0].transpose(1, 0, 2).reshape(
                P, KC * P)
            wpk[:, o1:o1 + 1024] = wr[:, :, 1].transpose(1, 0, 2).reshape(
                P, KC * P)
        ws = np.ascontiguousarray(np.asarray(Wv, np.float32)[rs].T)
        wpk[:, WV_OFF:WV_OFF + 2048] = ws.reshape(KC, P, C).transpose(
            1, 0, 2).reshape(P, KC * C)
        wps = np.ascontiguousarray(np.asarray(Wp, np.float32)[:, rs].T)
        wpk[:, WP_OFF:WP_OFF + 2048] = wps.reshape(2, P, D).transpose(
            1, 0, 2).reshape(P, 2 * D)
        wpk[:, TM_OFF:TM_OFF + P] = tm
        m["wpk"] = wpk.astype(bf16)
        bqs = np.asarray(bq, np.float32)[rs].reshape(2, P)
        bks = np.asarray(bk, np.float32)[rs].reshape(2, P)
        m["bqk"] = np.stack([bqs[0], bks[0], bqs[1], bks[1]], axis=1)
        m["bv"] = np.asarray(bv, np.float32)[rs].reshape(1, C)
        in_maps.append(m)
    return in_maps


def kernel(x, Wq, bq, Wk, bk, Wv, bv, Wp, bp, _trace=False):
    nc = _get_nc()
    in_maps = make_in_maps(x, Wq, bq, Wk, bk, Wv, bv, Wp, bp)
    res = bass_utils.run_bass_kernel_spmd(
        nc, in_maps, core_ids=list(range(N_CORES)), trace=_trace)
    kernel.last_exec_time_ns = res.exec_time_ns
    bp = np.asarray(bp, np.float32)
    out = np.empty((B, T, D), np.float32)
    for b in range(B):
        acc = np.zeros((T, D), np.float32)
        for hg in range(4):
            acc += res.results[4 * b + hg]["out"].astype(np.float32)
        out[b] = acc + bp
    return out
